# revision 10
# baseline (speedup 1.0000x reference)
"""AdaptiveLoss (co-teaching style loss) Trainium2 kernel, 8 NeuronCores.

Matches the jax reference:
  per-sample CE of y1,y2 at targets -> total_loss; symmetric batchmean KL
  between softmax(y1) and softmax(y2); clean mean over the num_remember
  globally-smallest total_loss; correction term over the noisy set
  (empty for prod_conf<=0.5, which the device flags with a sound filter).

Per core (data-parallel over N, 32768 rows = 16 macro-tiles [128,16,128]):
  ACT    : E = exp(T) f32->bf16, one op per macro-tensor
  DVE    : row maxes (packed reduce), bf16 products (T1-T2)*E with
           pair-halving adds, packed segmented reduces for s1,s2,A1,A2
  GPSIMD : D = T1-T2 (most macros), per-row target gathers (indirect_copy)
  kl_i = A1/s1 - A2/s2 ; total_loss_i = ln(s1)+ln(s2) - y1[t] - y2[t]

Global k-th smallest: 32-edge fixed grid counts (tensor_scalar+accum),
one AllReduce, exact below-edge count/sum at the picked edge, boundary
values extracted with sparse_gather; host sorts the tiny boundary set and
finishes the scalar (plus exact corr fix-up for flagged rows, and an
exact fallback from the dumped per-sample losses if the grid was missed).
"""

import numpy as np

N, C = 262144, 128
NCORES = 8
SHARD = N // NCORES            # 32768 rows per core
NT = SHARD // 128              # 256 row-tiles per core
BM = 16                        # tiles per macro-tile
NMACRO = NT // BM              # 16
EPOCHS = 100
CO_LAMBDA = 0.1
INCREMENT = 0.5 / EPOCHS

# selection grid: 32 dyadic edges over (SEL_LO, SEL_LO + 32*SEL_W]
SEL_LO = 12.9
SEL_W = 0.0625                 # 2^-4, exact in f32; span (12.9, 14.9]
SEL_NTH = 32
BV_CAP = 512                   # sparse_gather out free size (16*512 values)
GPS_D_MACROS = 16              # macros whose D runs on gpsimd (rest on DVE)

_CACHE = {}


def _row_index_map():
    """(p, t) -> local row index. Macro m covers rows [2048m, 2048(m+1));
    partition p holds rows 2048m + 16p + b; stats column t = m*BM + b."""
    p = np.arange(128)[:, None]
    t = np.arange(NT)[None, :]
    m = t // BM
    b = t % BM
    return (2048 * m + 16 * p + b).astype(np.int64)  # [128, NT]


import os
DISABLE = set(os.environ.get('KDISABLE', '').split(','))


def _build():
    import concourse.bass as bass
    import concourse.bacc as bacc
    import concourse.tile as tile
    import concourse.bass_isa as bass_isa
    from concourse import mybir

    f32 = mybir.dt.float32
    bf16 = mybir.dt.bfloat16
    u32 = mybir.dt.uint32
    u16 = mybir.dt.uint16
    Alu = mybir.AluOpType
    Act = mybir.ActivationFunctionType
    X = mybir.AxisListType.X

    nc = bacc.Bacc("TRN2", target_bir_lowering=False, debug=False,
                   num_devices=NCORES)

    y1 = nc.dram_tensor("y1s", [SHARD, C], f32, kind="ExternalInput").ap()
    y2 = nc.dram_tensor("y2s", [SHARD, C], f32, kind="ExternalInput").ap()
    idx_d = nc.dram_tensor("idx16", [128, NT], u16, kind="ExternalInput").ap()
    thr_d = nc.dram_tensor("thr", [128, SEL_NTH], f32, kind="ExternalInput").ap()
    kval_d = nc.dram_tensor("kval", [128, 1], f32, kind="ExternalInput").ap()

    o_tl = nc.dram_tensor("o_tl", [128, NT], f32, kind="ExternalOutput").ap()
    o_flags = nc.dram_tensor("o_flags", [128, NT], f32, kind="ExternalOutput").ap()
    o_misc = nc.dram_tensor("o_misc", [128, 8], f32, kind="ExternalOutput").ap()
    o_bv = nc.dram_tensor("o_bv", [16, BV_CAP], f32, kind="ExternalOutput").ap()
    o_nf = nc.dram_tensor("o_nf", [1, 8], u32, kind="ExternalOutput").ap()
    o_cnt = nc.dram_tensor("o_cnt", [1, SEL_NTH], f32, kind="ExternalOutput").ap()

    y1v = y1.rearrange("(m p b) c -> m p b c", m=NMACRO, p=128)
    y2v = y2.rearrange("(m p b) c -> m p b c", m=NMACRO, p=128)

    with tile.TileContext(nc) as tc:
        with (
            tc.tile_pool(name="io", bufs=3) as iop,
            tc.tile_pool(name="work", bufs=3) as wp,
            tc.tile_pool(name="half", bufs=2) as hp,
            tc.tile_pool(name="stats", bufs=1) as sp,
            tc.tile_pool(name="epi", bufs=1) as ep,
            tc.tile_pool(name="escr", bufs=2) as escr,
            tc.tile_pool(name="dram", bufs=1, space="DRAM") as dp,
        ):
            S1 = sp.tile([128, NT], f32, tag="S1")
            S2 = sp.tile([128, NT], f32, tag="S2")
            A1 = sp.tile([128, NT], f32, tag="A1")
            A2 = sp.tile([128, NT], f32, tag="A2")
            Y1T = sp.tile([128, NT], f32, tag="Y1T")
            Y2T = sp.tile([128, NT], f32, tag="Y2T")
            M1 = sp.tile([128, NT], f32, tag="M1")
            M2 = sp.tile([128, NT], f32, tag="M2")

            IDX = sp.tile([128, NT], u16, tag="IDX")
            thr = sp.tile([128, SEL_NTH], f32, tag="thr")
            kval = sp.tile([128, 1], f32, tag="kval")
            nc.sync.dma_start(out=IDX, in_=idx_d)
            nc.sync.dma_start(out=thr, in_=thr_d)
            nc.sync.dma_start(out=kval, in_=kval_d)

            # ---------------- streaming phase ----------------
            for m in range(NMACRO):
                ts = slice(m * BM, (m + 1) * BM)
                T1 = iop.tile([128, BM, C], f32, tag="T1")
                T2 = iop.tile([128, BM, C], f32, tag="T2")
                nc.sync.dma_start(out=T1, in_=y1v[m])
                nc.sync.dma_start(out=T2, in_=y2v[m])

                E1 = wp.tile([128, BM, C], bf16, tag="E1")
                E2 = wp.tile([128, BM, C], bf16, tag="E2")
                D = wp.tile([128, BM, C], bf16, tag="D")
                PD1 = wp.tile([128, BM, C], bf16, tag="PD1")
                PD2 = wp.tile([128, BM, C], bf16, tag="PD2")

                # exps (one big ACT op each)
                nc.scalar.activation(out=E1, in_=T1, func=Act.Exp)
                nc.scalar.activation(out=E2, in_=T2, func=Act.Exp)

                # D = T1 - T2 (bf16 out)
                if m < GPS_D_MACROS:
                    nc.gpsimd.tensor_tensor(out=D, in0=T1, in1=T2, op=Alu.subtract)
                else:
                    nc.vector.tensor_tensor(out=D, in0=T1, in1=T2, op=Alu.subtract)

                # target gathers: Y[:, t] = T[p, idx[p, t]] (gpsimd software)
                if "gather" in DISABLE:
                    nc.vector.memset(Y1T[:, ts], 5.0)
                    nc.vector.memset(Y2T[:, ts], 5.0)
                else:
                    nc.gpsimd.indirect_copy(
                        out=Y1T[:, ts], data=T1.rearrange("p a b -> p (a b)"),
                        idxs=IDX[:, ts], i_know_ap_gather_is_preferred=True)
                    nc.gpsimd.indirect_copy(
                        out=Y2T[:, ts], data=T2.rearrange("p a b -> p (a b)"),
                        idxs=IDX[:, ts], i_know_ap_gather_is_preferred=True)

                # per-row stat chains: bf16 pair-halving x2, then packed
                # reduce of [128,BM,32]
                def chain(dst, src, op):
                    H = hp.tile([128, BM, C // 2], bf16, tag="H")
                    nc.vector.tensor_tensor(
                        out=H, in0=src[:, :, 0:64], in1=src[:, :, 64:128], op=op)
                    Q = hp.tile([128, BM, C // 4], bf16, tag="Q")
                    nc.vector.tensor_tensor(
                        out=Q, in0=H[:, :, 0:32], in1=H[:, :, 32:64], op=op)
                    nc.vector.tensor_reduce(out=dst, in_=Q, axis=X, op=op)

                chain(S1[:, ts], E1, Alu.add)
                chain(S2[:, ts], E2, Alu.add)
                # maxes from E (monotone under bf16; flag margin covers it)
                chain(M1[:, ts], E1, Alu.max)
                chain(M2[:, ts], E2, Alu.max)

                # A1 = sum (T1-T2)*E1, A2 = sum (T1-T2)*E2
                nc.vector.tensor_tensor(out=PD1, in0=D, in1=E1, op=Alu.mult)
                nc.vector.tensor_tensor(out=PD2, in0=D, in1=E2, op=Alu.mult)
                chain(A1[:, ts], PD1, Alu.add)
                chain(A2[:, ts], PD2, Alu.add)

            # ---------------- epilogue ----------------
            MISC = ep.tile([128, 8], f32, tag="MISC")
            nc.vector.memset(MISC, 0.0)

            LZ1 = ep.tile([128, NT], f32, tag="LZ1")
            LZ2 = ep.tile([128, NT], f32, tag="LZ2")
            nc.scalar.activation(out=LZ1, in_=S1, func=Act.Ln)
            nc.scalar.activation(out=LZ2, in_=S2, func=Act.Ln)

            R1 = ep.tile([128, NT], f32, tag="R1")
            R2 = ep.tile([128, NT], f32, tag="R2")
            nc.vector.reciprocal(out=R1, in_=S1)
            nc.vector.reciprocal(out=R2, in_=S2)

            KA = ep.tile([128, NT], f32, tag="KA")
            KB = ep.tile([128, NT], f32, tag="KB")
            nc.vector.tensor_tensor(out=KA, in0=A1, in1=R1, op=Alu.mult)
            nc.vector.tensor_tensor(out=KB, in0=A2, in1=R2, op=Alu.mult)
            KL = ep.tile([128, NT], f32, tag="KL")
            nc.vector.tensor_tensor(out=KL, in0=KA, in1=KB, op=Alu.subtract)
            nc.vector.tensor_reduce(out=MISC[:, 2:3], in_=KL, axis=X, op=Alu.add)

            LZ12 = ep.tile([128, NT], f32, tag="LZ12")
            nc.vector.tensor_tensor(out=LZ12, in0=LZ1, in1=LZ2, op=Alu.add)
            Y12 = ep.tile([128, NT], f32, tag="Y12")
            nc.vector.tensor_tensor(out=Y12, in0=Y1T, in1=Y2T, op=Alu.add)
            TL = ep.tile([128, NT], f32, tag="TL")
            nc.vector.tensor_tensor(out=TL, in0=LZ12, in1=Y12, op=Alu.subtract)
            nc.vector.tensor_reduce(out=MISC[:, 3:4], in_=TL, axis=X, op=Alu.add)
            nc.sync.dma_start(out=o_tl, in_=TL)

            # conf flags (sound superset of prod_conf > 0.5 rows)
            ME1 = ep.tile([128, NT], f32, tag="ME1")
            ME2 = ep.tile([128, NT], f32, tag="ME2")
            nc.scalar.activation(out=ME1, in_=M1, func=Act.Ln)
            nc.scalar.activation(out=ME2, in_=M2, func=Act.Ln)
            M12 = ep.tile([128, NT], f32, tag="M12")
            nc.vector.tensor_tensor(out=M12, in0=ME1, in1=ME2, op=Alu.add)
            FLG0 = ep.tile([128, NT], f32, tag="FLG0")
            nc.vector.tensor_tensor(out=FLG0, in0=M12, in1=LZ12, op=Alu.subtract)
            FLAGS = ep.tile([128, NT], f32, tag="FLAGS")
            nc.vector.tensor_scalar(
                out=FLAGS, in0=FLG0, scalar1=float(np.log(0.5) - 0.02),
                scalar2=None, op0=Alu.is_gt)
            nc.vector.tensor_reduce(out=MISC[:, 5:6], in_=FLAGS, axis=X, op=Alu.add)
            nc.sync.dma_start(out=o_flags, in_=FLAGS)

            # --- distributed selection: counts vs fixed grid ---
            CNT = ep.tile([128, SEL_NTH], f32, tag="CNT")
            for j in range(SEL_NTH):
                cs = escr.tile([128, NT], f32, tag="cs")
                nc.vector.tensor_scalar(
                    out=cs, in0=TL, scalar1=thr[:, j:j + 1], scalar2=None,
                    op0=Alu.is_lt, op1=Alu.add, accum_out=CNT[:, j:j + 1])

            CNTP = ep.tile([128, SEL_NTH], f32, tag="CNTP")
            nc.gpsimd.partition_all_reduce(
                out_ap=CNTP, in_ap=CNT, channels=128,
                reduce_op=bass_isa.ReduceOp.add)

            cc_in = dp.tile([1, SEL_NTH], f32, tag="cc_in")
            cc_out = dp.tile([1, SEL_NTH], f32, tag="cc_out")
            nc.sync.dma_start(out=cc_in, in_=CNTP[0:1, :])
            nc.gpsimd.collective_compute(
                "AllReduce", Alu.add,
                replica_groups=[list(range(NCORES))],
                ins=[cc_in[:].opt()], outs=[cc_out[:].opt()])
            CNTG0 = ep.tile([1, SEL_NTH], f32, tag="CNTG0")
            nc.sync.dma_start(out=CNTG0, in_=cc_out)
            nc.sync.dma_start(out=o_cnt, in_=CNTG0)
            CNTG = ep.tile([128, SEL_NTH], f32, tag="CNTG")
            nc.gpsimd.partition_broadcast(out_ap=CNTG, in_ap=CNTG0, channels=128)

            # edge a = SEL_LO + s*W with s = #{j: cnt_j < k}
            EM = ep.tile([128, SEL_NTH], f32, tag="EM")
            nc.vector.tensor_scalar(
                out=EM, in0=CNTG, scalar1=kval[:, 0:1], scalar2=None,
                op0=Alu.is_lt)
            SIDX = ep.tile([128, 1], f32, tag="SIDX")
            nc.vector.tensor_reduce(out=SIDX, in_=EM, axis=X, op=Alu.add)
            AED = ep.tile([128, 1], f32, tag="AED")
            nc.vector.tensor_scalar(
                out=AED, in0=SIDX, scalar1=SEL_W, scalar2=SEL_LO,
                op0=Alu.mult, op1=Alu.add)
            AEDW = ep.tile([128, 1], f32, tag="AEDW")
            nc.vector.tensor_scalar(
                out=AEDW, in0=AED, scalar1=SEL_W, scalar2=None, op0=Alu.add)
            nc.vector.tensor_copy(out=MISC[:, 4:5], in_=AED)

            # exact n_below / S_below at edge a
            e1s = escr.tile([128, NT], f32, tag="cs")
            nc.vector.tensor_scalar(
                out=e1s, in0=TL, scalar1=AED[:, 0:1], scalar2=None,
                op0=Alu.is_lt, op1=Alu.add, accum_out=MISC[:, 0:1])
            e2s = escr.tile([128, NT], f32, tag="cs")
            nc.vector.scalar_tensor_tensor(
                out=e2s, in0=TL, scalar=AED[:, 0:1], in1=TL,
                op0=Alu.is_lt, op1=Alu.mult, accum_out=MISC[:, 1:2])

            # boundary values in [a, a+W) -> BV = tl, else -1
            MA = ep.tile([128, NT], f32, tag="MA")
            nc.vector.tensor_scalar(
                out=MA, in0=TL, scalar1=AED[:, 0:1], scalar2=None, op0=Alu.is_ge)
            MB = ep.tile([128, NT], f32, tag="MB")
            nc.vector.tensor_scalar(
                out=MB, in0=TL, scalar1=AEDW[:, 0:1], scalar2=None, op0=Alu.is_lt)
            MAB = ep.tile([128, NT], f32, tag="MAB")
            nc.vector.tensor_tensor(out=MAB, in0=MA, in1=MB, op=Alu.mult)
            TP1 = ep.tile([128, NT], f32, tag="TP1")
            nc.vector.tensor_scalar(
                out=TP1, in0=TL, scalar1=1.0, scalar2=None, op0=Alu.add)
            PRD = ep.tile([128, NT], f32, tag="PRD")
            nc.vector.tensor_tensor(out=PRD, in0=TP1, in1=MAB, op=Alu.mult)
            BV = ep.tile([128, NT], f32, tag="BV")
            nc.vector.tensor_scalar(
                out=BV, in0=PRD, scalar1=1.0, scalar2=None, op0=Alu.subtract)

            # repack to 16 partitions and extract
            BVO = ep.tile([16, BV_CAP], f32, tag="BVO")
            NF = ep.tile([1, 8], u32, tag="NF")
            nc.vector.memset(NF, 0)
            if "extract" in DISABLE:
                nc.vector.memset(BVO, -1.0)
            else:
                BV16 = ep.tile([16, 8 * NT], f32, tag="BV16")
                for ch in range(8):
                    nc.sync.dma_start(
                        out=BV16[:, ch * NT:(ch + 1) * NT],
                        in_=BV[16 * ch:16 * (ch + 1), :])
                for ch in range(8):
                    nc.gpsimd.sparse_gather(
                        out=BVO[:, ch * (BV_CAP // 8):(ch + 1) * (BV_CAP // 8)],
                        in_=BV16[:, ch * NT:(ch + 1) * NT],
                        num_found=NF[0:1, ch:ch + 1])

            nc.sync.dma_start(out=o_misc, in_=MISC)
            nc.sync.dma_start(out=o_bv, in_=BVO)
            nc.sync.dma_start(out=o_nf, in_=NF)

    nc.compile()
    return nc


def _get_compiled():
    if "nc" not in _CACHE:
        _CACHE["nc"] = _build()
    return _CACHE["nc"]


def _host_inputs(y1, y2, targets):
    idx = _row_index_map()                      # [128, NT] local rows
    b_of_t = (np.arange(NT) % BM)[None, :]      # group within macro
    thr_row = (np.arange(1, SEL_NTH + 1, dtype=np.float32)
               * np.float32(SEL_W) + np.float32(SEL_LO))
    thr = np.broadcast_to(thr_row[None, :], (128, SEL_NTH)).copy()

    in_maps = []
    for cid in range(NCORES):
        lo = cid * SHARD
        tshard = np.asarray(targets[lo:lo + SHARD]).astype(np.int64)
        tgt = tshard[idx]                       # [128, NT]
        idx16 = (b_of_t * C + tgt).astype(np.uint16)
        in_maps.append({
            "y1s": np.ascontiguousarray(y1[lo:lo + SHARD]),
            "y2s": np.ascontiguousarray(y2[lo:lo + SHARD]),
            "idx16": idx16,
            "thr": thr,
            "kval": np.zeros((128, 1), np.float32),
        })
    return in_maps


def _host_finish(results, y1, y2, targets, epoch, k):
    n = N
    idx = _row_index_map()

    kl_sum = np.float64(0.0)
    s_total = np.float64(0.0)
    n_below = np.float64(0.0)
    s_below = np.float64(0.0)
    boundary = []
    flags_rows = []
    tl_full = np.empty(n, np.float32)
    fallback = False
    edge_a = None

    for cid, r in enumerate(results):
        misc = r["o_misc"].astype(np.float64)        # [128, 8]
        kl_sum += misc[:, 2].sum()
        s_total += misc[:, 3].sum()
        n_below += misc[:, 0].sum()
        s_below += misc[:, 1].sum()
        ea = r["o_misc"][0, 4]
        if edge_a is None:
            edge_a = ea
        elif ea != edge_a:
            fallback = True
        bvo = r["o_bv"]                              # [16, BV_CAP] in 8 chunks
        w = BV_CAP // 8
        for ch in range(8):
            cnt = int(r["o_nf"][0, ch])
            blk = bvo[:, ch * w:(ch + 1) * w]
            if cnt > blk.size:
                fallback = True
                cnt = 0
            boundary.append(blk.reshape(-1, order="F")[:cnt])
        tl_core = r["o_tl"]                          # [128, NT]
        gl = cid * SHARD + idx
        tl_full[gl.ravel()] = tl_core.ravel()
        if r["o_misc"][:, 5].sum() > 0:
            fl = r["o_flags"] > 0.5
            flags_rows.extend((cid * SHARD + idx[fl]).tolist())

    boundary = np.concatenate(boundary) if boundary else np.empty(0, np.float32)

    if epoch == 0:
        return np.float32(s_total / n)

    need = k - int(round(n_below))
    if fallback or need < 0 or need > boundary.size:
        # safety net: exact selection on the dumped per-sample losses
        part = np.partition(tl_full, k - 1)
        tau = part[k - 1]
        below = tl_full < tau
        nb = int(below.sum())
        clean_sum = np.float64(tl_full[below].sum()) + (k - nb) * np.float64(tau)
    else:
        bs = np.sort(boundary)
        sel = bs[:need]
        tau = sel[-1] if need > 0 else np.float32(edge_a)
        clean_sum = s_below + np.float64(sel.sum())

    clean_mean = clean_sum / k

    # corr term: only rows with prod_conf > 0.5 can contribute; device
    # flags are a sound superset of those rows (approx but margin-padded).
    corr_mean = np.float64(0.0)
    if flags_rows:
        rows = np.array(sorted(set(flags_rows)), dtype=np.int64)
        corr_vals = []
        cnt_mask = 0
        for rr in rows:
            v = tl_full[rr]
            if v < tau:
                continue
            if v == tau:
                tie_rows = np.nonzero(tl_full == tau)[0]
                nb_strict = int((tl_full < tau).sum())
                n_clean_ties = k - nb_strict
                pos = int(np.searchsorted(tie_rows, rr))
                if pos < n_clean_ties:
                    continue
            a1 = y1[rr].astype(np.float64)
            a2 = y2[rr].astype(np.float64)
            e1 = np.exp(a1 - a1.max())
            p1 = e1 / e1.sum()
            e2 = np.exp(a2 - a2.max())
            p2 = e2 / e2.sum()
            pr1 = int(np.argmax(a1))
            pr2 = int(np.argmax(a2))
            conf = p1.max() * p2.max()
            if pr1 == pr2 and conf > 0.5:
                w = np.sqrt(conf)
                corr_vals.append(w * (-np.log(p1[pr1]) - np.log(p2[pr1])))
                cnt_mask += 1
        if cnt_mask > 0:
            corr_mean = np.float64(np.sum(corr_vals)) / cnt_mask

    kl_loss = kl_sum / n
    return np.float32(clean_mean + corr_mean + CO_LAMBDA * kl_loss)


def kernel(**inputs):
    from concourse import bass_utils

    y1 = np.asarray(inputs["y1"], dtype=np.float32)
    y2 = np.asarray(inputs["y2"], dtype=np.float32)
    targets = np.asarray(inputs["targets"])
    epoch = int(np.asarray(inputs["epoch"]))

    forget_rate = min(0.5, INCREMENT * epoch)
    remember_rate = max(0.5, 1.0 - forget_rate)
    k = int(remember_rate * N)

    nc = _get_compiled()
    in_maps = _host_inputs(y1, y2, targets)
    for m in in_maps:
        m["kval"][:] = np.float32(k)

    res = bass_utils.run_bass_kernel_spmd(
        nc, in_maps, core_ids=list(range(NCORES)))
    results = res.results

    return np.array(_host_finish(results, y1, y2, targets, epoch, k),
                    dtype=np.float32)


# revision 16
# speedup vs baseline: 1.1901x; 1.1901x over previous
"""AdaptiveLoss (co-teaching style loss) Trainium2 kernel, 8 NeuronCores.

Matches the jax reference:
  per-sample CE of y1,y2 at targets -> total_loss; symmetric batchmean KL
  between softmax(y1) and softmax(y2); clean mean over the num_remember
  globally-smallest total_loss; correction term over the noisy set
  (empty for prod_conf<=0.5, which the device flags with a sound filter).

Per core (data-parallel over N, 32768 rows = 16 macro-tiles [128,16,128]):
  ACT    : E = exp(T) f32->bf16, one op per macro-tensor
  DVE    : row maxes (packed reduce), bf16 products (T1-T2)*E with
           pair-halving adds, packed segmented reduces for s1,s2,A1,A2
  GPSIMD : D = T1-T2 (most macros), per-row target gathers (indirect_copy)
  kl_i = A1/s1 - A2/s2 ; total_loss_i = ln(s1)+ln(s2) - y1[t] - y2[t]

Global k-th smallest: 32-edge fixed grid counts (tensor_scalar+accum),
one AllReduce, exact below-edge count/sum at the picked edge, boundary
values extracted with sparse_gather; host sorts the tiny boundary set and
finishes the scalar (plus exact corr fix-up for flagged rows, and an
exact fallback from the dumped per-sample losses if the grid was missed).
"""

import numpy as np

N, C = 262144, 128
NCORES = 8
SHARD = N // NCORES            # 32768 rows per core
NT = SHARD // 128              # 256 row-tiles per core
BM = 16                        # tiles per macro-tile
NMACRO = NT // BM              # 16
EPOCHS = 100
CO_LAMBDA = 0.1
INCREMENT = 0.5 / EPOCHS

# selection grid: 32 dyadic edges over (SEL_LO, SEL_LO + 32*SEL_W]
SEL_LO = 12.9
SEL_W = 0.0625                 # 2^-4, exact in f32; span (12.9, 14.9]
SEL_NTH = 32
BV_CAP = 512                   # sparse_gather out free size (16*512 values)
GPS_D_MACROS = 16              # macros whose D runs on gpsimd (rest on DVE)

_CACHE = {}


def _row_index_map():
    """(p, t) -> local row index. Macro m covers rows [2048m, 2048(m+1));
    partition p holds rows 2048m + 16p + b; stats column t = m*BM + b."""
    p = np.arange(128)[:, None]
    t = np.arange(NT)[None, :]
    m = t // BM
    b = t % BM
    return (2048 * m + 16 * p + b).astype(np.int64)  # [128, NT]


import os
DISABLE = set(os.environ.get('KDISABLE', '').split(','))


def _build():
    import concourse.bass as bass
    import concourse.bacc as bacc
    import concourse.tile as tile
    import concourse.bass_isa as bass_isa
    from concourse import mybir

    f32 = mybir.dt.float32
    bf16 = mybir.dt.bfloat16
    u32 = mybir.dt.uint32
    u16 = mybir.dt.uint16
    Alu = mybir.AluOpType
    Act = mybir.ActivationFunctionType
    X = mybir.AxisListType.X

    nc = bacc.Bacc("TRN2", target_bir_lowering=False, debug=False,
                   num_devices=NCORES)

    y1 = nc.dram_tensor("y1s", [SHARD, C], f32, kind="ExternalInput").ap()
    y2 = nc.dram_tensor("y2s", [SHARD, C], f32, kind="ExternalInput").ap()
    idx_d = nc.dram_tensor("idx16", [128, NT], u16, kind="ExternalInput").ap()
    thr_d = nc.dram_tensor("thr", [128, SEL_NTH], f32, kind="ExternalInput").ap()
    kval_d = nc.dram_tensor("kval", [128, 1], f32, kind="ExternalInput").ap()

    o_tl = nc.dram_tensor("o_tl", [128, NT], f32, kind="ExternalOutput").ap()
    o_misc = nc.dram_tensor("o_misc", [128, 8], f32, kind="ExternalOutput").ap()
    o_bv = nc.dram_tensor("o_bv", [16, BV_CAP], f32, kind="ExternalOutput").ap()
    o_nf = nc.dram_tensor("o_nf", [1, 8], u32, kind="ExternalOutput").ap()
    o_cnt = nc.dram_tensor("o_cnt", [1, SEL_NTH], f32, kind="ExternalOutput").ap()

    y1v = y1.rearrange("(m p b) c -> m p b c", m=NMACRO, p=128)
    y2v = y2.rearrange("(m p b) c -> m p b c", m=NMACRO, p=128)

    with tile.TileContext(nc) as tc:
        with (
            tc.tile_pool(name="io", bufs=3) as iop,
            tc.tile_pool(name="work", bufs=3) as wp,
            tc.tile_pool(name="half", bufs=2) as hp,
            tc.tile_pool(name="stats", bufs=1) as sp,
            tc.tile_pool(name="epi", bufs=1) as ep,
            tc.tile_pool(name="escr", bufs=2) as escr,
            tc.tile_pool(name="dram", bufs=1, space="DRAM") as dp,
        ):
            S1 = sp.tile([128, NT], f32, tag="S1")
            S2 = sp.tile([128, NT], f32, tag="S2")
            A1 = sp.tile([128, NT], f32, tag="A1")
            A2 = sp.tile([128, NT], f32, tag="A2")
            Y1T = sp.tile([128, NT], f32, tag="Y1T")
            Y2T = sp.tile([128, NT], f32, tag="Y2T")
            IDX = sp.tile([128, NT], u16, tag="IDX")
            thr = sp.tile([128, SEL_NTH], f32, tag="thr")
            kval = sp.tile([128, 1], f32, tag="kval")
            nc.sync.dma_start(out=IDX, in_=idx_d)
            nc.sync.dma_start(out=thr, in_=thr_d)
            nc.sync.dma_start(out=kval, in_=kval_d)

            # ---------------- streaming phase ----------------
            for m in range(NMACRO):
                ts = slice(m * BM, (m + 1) * BM)
                T1 = iop.tile([128, BM, C], f32, tag="T1")
                T2 = iop.tile([128, BM, C], f32, tag="T2")
                nc.sync.dma_start(out=T1, in_=y1v[m])
                nc.sync.dma_start(out=T2, in_=y2v[m])

                E1 = wp.tile([128, BM, C], bf16, tag="E1")
                E2 = wp.tile([128, BM, C], bf16, tag="E2")
                D = wp.tile([128, BM, C], bf16, tag="D")
                PD1 = wp.tile([128, BM, C], bf16, tag="PD1")
                PD2 = wp.tile([128, BM, C], bf16, tag="PD2")

                # exps (one big ACT op each)
                nc.scalar.activation(out=E1, in_=T1, func=Act.Exp)
                nc.scalar.activation(out=E2, in_=T2, func=Act.Exp)

                # D = T1 - T2 (bf16 out)
                if m < GPS_D_MACROS:
                    nc.gpsimd.tensor_tensor(out=D, in0=T1, in1=T2, op=Alu.subtract)
                else:
                    nc.vector.tensor_tensor(out=D, in0=T1, in1=T2, op=Alu.subtract)

                # target gathers: Y[:, t] = T[p, idx[p, t]] (gpsimd software)
                if "gather" in DISABLE:
                    nc.vector.memset(Y1T[:, ts], 5.0)
                    nc.vector.memset(Y2T[:, ts], 5.0)
                else:
                    nc.gpsimd.indirect_copy(
                        out=Y1T[:, ts], data=T1.rearrange("p a b -> p (a b)"),
                        idxs=IDX[:, ts], i_know_ap_gather_is_preferred=True)
                    nc.gpsimd.indirect_copy(
                        out=Y2T[:, ts], data=T2.rearrange("p a b -> p (a b)"),
                        idxs=IDX[:, ts], i_know_ap_gather_is_preferred=True)

                # per-row stat chains: one bf16 pair-halving + packed reduce
                def chain(dst, src, op):
                    H = hp.tile([128, BM, C // 2], bf16, tag="H")
                    nc.vector.tensor_tensor(
                        out=H, in0=src[:, :, 0:64], in1=src[:, :, 64:128], op=op)
                    nc.vector.tensor_reduce(out=dst, in_=H, axis=X, op=op)

                chain(S1[:, ts], E1, Alu.add)
                chain(S2[:, ts], E2, Alu.add)

                # A1 = sum (T1-T2)*E1, A2 = sum (T1-T2)*E2
                nc.vector.tensor_tensor(out=PD1, in0=D, in1=E1, op=Alu.mult)
                nc.vector.tensor_tensor(out=PD2, in0=D, in1=E2, op=Alu.mult)
                chain(A1[:, ts], PD1, Alu.add)
                chain(A2[:, ts], PD2, Alu.add)

            # ---------------- epilogue ----------------
            # Order matters per-engine: the selection counts go first so the
            # AllReduce launches ASAP; KL math and dumps fill its latency.
            MISC = ep.tile([128, 8], f32, tag="MISC")
            nc.vector.memset(MISC, 0.0)

            LZ1 = ep.tile([128, NT], f32, tag="LZ1")
            LZ2 = ep.tile([128, NT], f32, tag="LZ2")
            nc.scalar.activation(out=LZ1, in_=S1, func=Act.Ln)
            nc.scalar.activation(out=LZ2, in_=S2, func=Act.Ln)
            LZ12 = ep.tile([128, NT], f32, tag="LZ12")
            nc.vector.tensor_tensor(out=LZ12, in0=LZ1, in1=LZ2, op=Alu.add)
            Y12 = ep.tile([128, NT], f32, tag="Y12")
            nc.vector.tensor_tensor(out=Y12, in0=Y1T, in1=Y2T, op=Alu.add)
            TL = ep.tile([128, NT], f32, tag="TL")
            nc.vector.tensor_tensor(out=TL, in0=LZ12, in1=Y12, op=Alu.subtract)

            # --- distributed selection: counts vs fixed grid ---
            CNT = ep.tile([128, SEL_NTH], f32, tag="CNT")
            for j in range(SEL_NTH):
                cs = escr.tile([128, NT], f32, tag="cs")
                nc.vector.tensor_scalar(
                    out=cs, in0=TL, scalar1=thr[:, j:j + 1], scalar2=None,
                    op0=Alu.is_lt, op1=Alu.add, accum_out=CNT[:, j:j + 1])

            CNTP = ep.tile([128, SEL_NTH], f32, tag="CNTP")
            nc.gpsimd.partition_all_reduce(
                out_ap=CNTP, in_ap=CNT, channels=128,
                reduce_op=bass_isa.ReduceOp.add)

            cc_in = dp.tile([1, SEL_NTH], f32, tag="cc_in")
            cc_out = dp.tile([1, SEL_NTH], f32, tag="cc_out")
            nc.sync.dma_start(out=cc_in, in_=CNTP[0:1, :])
            nc.gpsimd.collective_compute(
                "AllReduce", Alu.add,
                replica_groups=[list(range(NCORES))],
                ins=[cc_in[:].opt()], outs=[cc_out[:].opt()])

            # CC-independent work fills the collective latency
            nc.sync.dma_start(out=o_tl, in_=TL)
            nc.vector.tensor_reduce(out=MISC[:, 3:4], in_=TL, axis=X, op=Alu.add)
            R1 = ep.tile([128, NT], f32, tag="R1")
            R2 = ep.tile([128, NT], f32, tag="R2")
            nc.vector.reciprocal(out=R1, in_=S1)
            nc.vector.reciprocal(out=R2, in_=S2)
            KA = ep.tile([128, NT], f32, tag="KA")
            KB = ep.tile([128, NT], f32, tag="KB")
            nc.vector.tensor_tensor(out=KA, in0=A1, in1=R1, op=Alu.mult)
            nc.vector.tensor_tensor(out=KB, in0=A2, in1=R2, op=Alu.mult)
            KL = ep.tile([128, NT], f32, tag="KL")
            nc.vector.tensor_tensor(out=KL, in0=KA, in1=KB, op=Alu.subtract)
            nc.vector.tensor_reduce(out=MISC[:, 2:3], in_=KL, axis=X, op=Alu.add)
            TP1 = ep.tile([128, NT], f32, tag="TP1")
            nc.vector.tensor_scalar(
                out=TP1, in0=TL, scalar1=1.0, scalar2=None, op0=Alu.add)

            CNTG0 = ep.tile([1, SEL_NTH], f32, tag="CNTG0")
            nc.sync.dma_start(out=CNTG0, in_=cc_out)
            nc.sync.dma_start(out=o_cnt, in_=CNTG0)
            CNTG = ep.tile([128, SEL_NTH], f32, tag="CNTG")
            nc.gpsimd.partition_broadcast(out_ap=CNTG, in_ap=CNTG0, channels=128)

            # edge a = SEL_LO + s*W with s = #{j: cnt_j < k}
            EM = ep.tile([128, SEL_NTH], f32, tag="EM")
            nc.vector.tensor_scalar(
                out=EM, in0=CNTG, scalar1=kval[:, 0:1], scalar2=None,
                op0=Alu.is_lt)
            SIDX = ep.tile([128, 1], f32, tag="SIDX")
            nc.vector.tensor_reduce(out=SIDX, in_=EM, axis=X, op=Alu.add)
            AED = ep.tile([128, 1], f32, tag="AED")
            nc.vector.tensor_scalar(
                out=AED, in0=SIDX, scalar1=SEL_W, scalar2=SEL_LO,
                op0=Alu.mult, op1=Alu.add)
            AEDW = ep.tile([128, 1], f32, tag="AEDW")
            nc.vector.tensor_scalar(
                out=AEDW, in0=AED, scalar1=SEL_W, scalar2=None, op0=Alu.add)
            nc.vector.tensor_copy(out=MISC[:, 4:5], in_=AED)

            # exact n_below / S_below at edge a
            e1s = escr.tile([128, NT], f32, tag="cs")
            nc.vector.tensor_scalar(
                out=e1s, in0=TL, scalar1=AED[:, 0:1], scalar2=None,
                op0=Alu.is_lt, op1=Alu.add, accum_out=MISC[:, 0:1])
            e2s = escr.tile([128, NT], f32, tag="cs")
            nc.vector.scalar_tensor_tensor(
                out=e2s, in0=TL, scalar=AED[:, 0:1], in1=TL,
                op0=Alu.is_lt, op1=Alu.mult, accum_out=MISC[:, 1:2])

            # boundary values in [a, a+W) -> BV = tl, else -1
            MA = ep.tile([128, NT], f32, tag="MA")
            nc.vector.tensor_scalar(
                out=MA, in0=TL, scalar1=AED[:, 0:1], scalar2=None, op0=Alu.is_ge)
            MB = ep.tile([128, NT], f32, tag="MB")
            nc.vector.tensor_scalar(
                out=MB, in0=TL, scalar1=AEDW[:, 0:1], scalar2=None, op0=Alu.is_lt)
            MAB = ep.tile([128, NT], f32, tag="MAB")
            nc.vector.tensor_tensor(out=MAB, in0=MA, in1=MB, op=Alu.mult)
            PRD = ep.tile([128, NT], f32, tag="PRD")
            nc.vector.tensor_tensor(out=PRD, in0=TP1, in1=MAB, op=Alu.mult)
            BV = ep.tile([128, NT], f32, tag="BV")
            nc.vector.tensor_scalar(
                out=BV, in0=PRD, scalar1=1.0, scalar2=None, op0=Alu.subtract)

            # repack to 16 partitions and extract
            BVO = ep.tile([16, BV_CAP], f32, tag="BVO")
            NF = ep.tile([1, 8], u32, tag="NF")
            nc.vector.memset(NF, 0)
            if "extract" in DISABLE:
                nc.vector.memset(BVO, -1.0)
            else:
                BV16 = ep.tile([16, 8 * NT], f32, tag="BV16")
                for ch in range(8):
                    nc.sync.dma_start(
                        out=BV16[:, ch * NT:(ch + 1) * NT],
                        in_=BV[16 * ch:16 * (ch + 1), :])
                for ch in range(8):
                    nc.gpsimd.sparse_gather(
                        out=BVO[:, ch * (BV_CAP // 8):(ch + 1) * (BV_CAP // 8)],
                        in_=BV16[:, ch * NT:(ch + 1) * NT],
                        num_found=NF[0:1, ch:ch + 1])

            nc.sync.dma_start(out=o_misc, in_=MISC)
            nc.sync.dma_start(out=o_bv, in_=BVO)
            nc.sync.dma_start(out=o_nf, in_=NF)

    nc.compile()
    return nc


def _get_compiled():
    if "nc" not in _CACHE:
        _CACHE["nc"] = _build()
    return _CACHE["nc"]


def _host_inputs(y1, y2, targets):
    idx = _row_index_map()                      # [128, NT] local rows
    b_of_t = (np.arange(NT) % BM)[None, :]      # group within macro
    thr_row = (np.arange(1, SEL_NTH + 1, dtype=np.float32)
               * np.float32(SEL_W) + np.float32(SEL_LO))
    thr = np.broadcast_to(thr_row[None, :], (128, SEL_NTH)).copy()

    in_maps = []
    for cid in range(NCORES):
        lo = cid * SHARD
        tshard = np.asarray(targets[lo:lo + SHARD]).astype(np.int64)
        tgt = tshard[idx]                       # [128, NT]
        idx16 = (b_of_t * C + tgt).astype(np.uint16)
        in_maps.append({
            "y1s": np.ascontiguousarray(y1[lo:lo + SHARD]),
            "y2s": np.ascontiguousarray(y2[lo:lo + SHARD]),
            "idx16": idx16,
            "thr": thr,
            "kval": np.zeros((128, 1), np.float32),
        })
    return in_maps


def _host_finish(results, y1, y2, targets, epoch, k):
    n = N
    idx = _row_index_map()

    kl_sum = np.float64(0.0)
    s_total = np.float64(0.0)
    n_below = np.float64(0.0)
    s_below = np.float64(0.0)
    boundary = []
    tl_full = np.empty(n, np.float32)
    fallback = False
    edge_a = None

    for cid, r in enumerate(results):
        misc = r["o_misc"].astype(np.float64)        # [128, 8]
        kl_sum += misc[:, 2].sum()
        s_total += misc[:, 3].sum()
        n_below += misc[:, 0].sum()
        s_below += misc[:, 1].sum()
        ea = r["o_misc"][0, 4]
        if edge_a is None:
            edge_a = ea
        elif ea != edge_a:
            fallback = True
        bvo = r["o_bv"]                              # [16, BV_CAP] in 8 chunks
        w = BV_CAP // 8
        for ch in range(8):
            cnt = int(r["o_nf"][0, ch])
            blk = bvo[:, ch * w:(ch + 1) * w]
            if cnt > blk.size:
                fallback = True
                cnt = 0
            boundary.append(blk.reshape(-1, order="F")[:cnt])
        tl_core = r["o_tl"]                          # [128, NT]
        gl = cid * SHARD + idx
        tl_full[gl.ravel()] = tl_core.ravel()

    boundary = np.concatenate(boundary) if boundary else np.empty(0, np.float32)

    if epoch == 0:
        return np.float32(s_total / n)

    need = k - int(round(n_below))
    if fallback or need < 0 or need > boundary.size:
        # safety net: exact selection on the dumped per-sample losses
        part = np.partition(tl_full, k - 1)
        tau = part[k - 1]
        below = tl_full < tau
        nb = int(below.sum())
        clean_sum = np.float64(tl_full[below].sum()) + (k - nb) * np.float64(tau)
    else:
        bs = np.sort(boundary)
        sel = bs[:need]
        tau = sel[-1] if need > 0 else np.float32(edge_a)
        clean_sum = s_below + np.float64(sel.sum())

    clean_mean = clean_sum / k

    # corr term over the noisy set. Noisy rows all satisfy tl >= tau, a
    # tiny fraction of N; evaluate their agree/conf masks vectorized.
    corr_mean = np.float64(0.0)
    cand = np.nonzero(tl_full >= tau)[0]
    if cand.size:
        # resolve which candidates are actually noisy (stable-sort ties)
        vc = tl_full[cand]
        noisy_mask = vc > tau
        ties = np.nonzero(vc == tau)[0]
        if ties.size:
            nb_strict = int((tl_full < tau).sum())
            n_clean_ties = k - nb_strict
            tie_rows_all = np.nonzero(tl_full == tau)[0]
            pos = np.searchsorted(tie_rows_all, cand[ties])
            noisy_mask[ties] = pos >= n_clean_ties
        rows = cand[noisy_mask]
        if rows.size:
            a1 = y1[rows].astype(np.float64)
            a2 = y2[rows].astype(np.float64)
            m1 = a1.max(axis=1, keepdims=True)
            m2 = a2.max(axis=1, keepdims=True)
            e1 = np.exp(a1 - m1)
            e2 = np.exp(a2 - m2)
            s1 = e1.sum(axis=1, keepdims=True)
            s2 = e2.sum(axis=1, keepdims=True)
            p1 = e1 / s1
            p2 = e2 / s2
            pr1 = np.argmax(a1, axis=1)
            pr2 = np.argmax(a2, axis=1)
            conf = p1.max(axis=1) * p2.max(axis=1)
            mask = (pr1 == pr2) & (conf > 0.5)
            if mask.any():
                w = np.sqrt(conf[mask])
                sel1 = p1[mask, pr1[mask]]
                sel2 = p2[mask, pr1[mask]]
                corr = w * (-np.log(sel1) - np.log(sel2))
                corr_mean = np.float64(corr.sum()) / int(mask.sum())

    kl_loss = kl_sum / n
    return np.float32(clean_mean + corr_mean + CO_LAMBDA * kl_loss)


def kernel(**inputs):
    from concourse import bass_utils

    y1 = np.asarray(inputs["y1"], dtype=np.float32)
    y2 = np.asarray(inputs["y2"], dtype=np.float32)
    targets = np.asarray(inputs["targets"])
    epoch = int(np.asarray(inputs["epoch"]))

    forget_rate = min(0.5, INCREMENT * epoch)
    remember_rate = max(0.5, 1.0 - forget_rate)
    k = int(remember_rate * N)

    nc = _get_compiled()
    in_maps = _host_inputs(y1, y2, targets)
    for m in in_maps:
        m["kval"][:] = np.float32(k)

    res = bass_utils.run_bass_kernel_spmd(
        nc, in_maps, core_ids=list(range(NCORES)))
    results = res.results

    return np.array(_host_finish(results, y1, y2, targets, epoch, k),
                    dtype=np.float32)


# revision 17
# speedup vs baseline: 1.3373x; 1.1237x over previous
"""AdaptiveLoss (co-teaching style loss) Trainium2 kernel, 8 NeuronCores.

Matches the jax reference:
  per-sample CE of y1,y2 at targets -> total_loss; symmetric batchmean KL
  between softmax(y1) and softmax(y2); clean mean over the num_remember
  globally-smallest total_loss; correction term over the noisy set
  (empty for prod_conf<=0.5, which the device flags with a sound filter).

Per core (data-parallel over N, 32768 rows = 16 macro-tiles [128,16,128]):
  ACT    : E = exp(T) f32->bf16, one op per macro-tensor
  DVE    : row maxes (packed reduce), bf16 products (T1-T2)*E with
           pair-halving adds, packed segmented reduces for s1,s2,A1,A2
  GPSIMD : D = T1-T2 (most macros), per-row target gathers (indirect_copy)
  kl_i = A1/s1 - A2/s2 ; total_loss_i = ln(s1)+ln(s2) - y1[t] - y2[t]

Global k-th smallest: 32-edge fixed grid counts (tensor_scalar+accum),
one AllReduce, exact below-edge count/sum at the picked edge, boundary
values extracted with sparse_gather; host sorts the tiny boundary set and
finishes the scalar (plus exact corr fix-up for flagged rows, and an
exact fallback from the dumped per-sample losses if the grid was missed).
"""

import numpy as np

N, C = 262144, 128
NCORES = 8
SHARD = N // NCORES            # 32768 rows per core
NT = SHARD // 128              # 256 row-tiles per core
BM = 16                        # tiles per macro-tile
NMACRO = NT // BM              # 16
EPOCHS = 100
CO_LAMBDA = 0.1
INCREMENT = 0.5 / EPOCHS

# selection grid: 32 dyadic edges over (SEL_LO, SEL_LO + 32*SEL_W]
SEL_LO = 12.9
SEL_W = 0.125                  # 2^-3, exact in f32; span (12.9, 14.9]
SEL_NTH = 16
BV_CAP = 512                   # sparse_gather out free size (16*512 values)
GPS_D_MACROS = 16              # macros whose D runs on gpsimd (rest on DVE)

_CACHE = {}


def _row_index_map():
    """(p, t) -> local row index. Macro m covers rows [2048m, 2048(m+1));
    partition p holds rows 2048m + 16p + b; stats column t = m*BM + b."""
    p = np.arange(128)[:, None]
    t = np.arange(NT)[None, :]
    m = t // BM
    b = t % BM
    return (2048 * m + 16 * p + b).astype(np.int64)  # [128, NT]


import os
DISABLE = set(os.environ.get('KDISABLE', '').split(','))


def _build():
    import concourse.bass as bass
    import concourse.bacc as bacc
    import concourse.tile as tile
    import concourse.bass_isa as bass_isa
    from concourse import mybir

    f32 = mybir.dt.float32
    bf16 = mybir.dt.bfloat16
    u32 = mybir.dt.uint32
    u16 = mybir.dt.uint16
    Alu = mybir.AluOpType
    Act = mybir.ActivationFunctionType
    X = mybir.AxisListType.X

    nc = bacc.Bacc("TRN2", target_bir_lowering=False, debug=False,
                   num_devices=NCORES)

    y1 = nc.dram_tensor("y1s", [SHARD, C], f32, kind="ExternalInput").ap()
    y2 = nc.dram_tensor("y2s", [SHARD, C], f32, kind="ExternalInput").ap()
    idx_d = nc.dram_tensor("idx16", [128, NT], u16, kind="ExternalInput").ap()
    thr_d = nc.dram_tensor("thr", [128, SEL_NTH], f32, kind="ExternalInput").ap()
    kval_d = nc.dram_tensor("kval", [128, 1], f32, kind="ExternalInput").ap()

    o_tl = nc.dram_tensor("o_tl", [128, NT], f32, kind="ExternalOutput").ap()
    o_misc = nc.dram_tensor("o_misc", [128, 8], f32, kind="ExternalOutput").ap()
    o_cnt = nc.dram_tensor("o_cnt", [1, SEL_NTH], f32, kind="ExternalOutput").ap()

    y1v = y1.rearrange("(m p b) c -> m p b c", m=NMACRO, p=128)
    y2v = y2.rearrange("(m p b) c -> m p b c", m=NMACRO, p=128)

    with tile.TileContext(nc) as tc:
        with (
            tc.tile_pool(name="io", bufs=3) as iop,
            tc.tile_pool(name="work", bufs=3) as wp,
            tc.tile_pool(name="half", bufs=4) as hp,
            tc.tile_pool(name="stats", bufs=1) as sp,
            tc.tile_pool(name="epi", bufs=1) as ep,
            tc.tile_pool(name="escr", bufs=2) as escr,
            tc.tile_pool(name="dram", bufs=1, space="DRAM") as dp,
        ):
            S1 = sp.tile([128, NT], f32, tag="S1")
            S2 = sp.tile([128, NT], f32, tag="S2")
            A1 = sp.tile([128, NT], f32, tag="A1")
            A2 = sp.tile([128, NT], f32, tag="A2")
            Y1T = sp.tile([128, NT], f32, tag="Y1T")
            Y2T = sp.tile([128, NT], f32, tag="Y2T")
            IDX = sp.tile([128, NT], u16, tag="IDX")
            thr = sp.tile([128, SEL_NTH], f32, tag="thr")
            kval = sp.tile([128, 1], f32, tag="kval")
            nc.sync.dma_start(out=IDX, in_=idx_d)
            nc.sync.dma_start(out=thr, in_=thr_d)
            nc.sync.dma_start(out=kval, in_=kval_d)

            # ---------------- streaming phase ----------------
            for m in range(NMACRO):
                ts = slice(m * BM, (m + 1) * BM)
                T1 = iop.tile([128, BM, C], f32, tag="T1")
                T2 = iop.tile([128, BM, C], f32, tag="T2")
                nc.sync.dma_start(out=T1, in_=y1v[m])
                nc.sync.dma_start(out=T2, in_=y2v[m])

                E1 = wp.tile([128, BM, C], bf16, tag="E1")
                E2 = wp.tile([128, BM, C], bf16, tag="E2")
                D = wp.tile([128, BM, C], bf16, tag="D")
                PD1 = wp.tile([128, BM, C], bf16, tag="PD1")
                PD2 = wp.tile([128, BM, C], bf16, tag="PD2")

                # exps (one big ACT op each)
                nc.scalar.activation(out=E1, in_=T1, func=Act.Exp)
                nc.scalar.activation(out=E2, in_=T2, func=Act.Exp)

                # D = T1 - T2 (bf16 out)
                if m < GPS_D_MACROS:
                    nc.gpsimd.tensor_tensor(out=D, in0=T1, in1=T2, op=Alu.subtract)
                else:
                    nc.vector.tensor_tensor(out=D, in0=T1, in1=T2, op=Alu.subtract)

                # target gathers: Y[:, t] = T[p, idx[p, t]] (gpsimd software)
                if "gather" in DISABLE:
                    nc.vector.memset(Y1T[:, ts], 5.0)
                    nc.vector.memset(Y2T[:, ts], 5.0)
                else:
                    nc.gpsimd.indirect_copy(
                        out=Y1T[:, ts], data=T1.rearrange("p a b -> p (a b)"),
                        idxs=IDX[:, ts], i_know_ap_gather_is_preferred=True)
                    nc.gpsimd.indirect_copy(
                        out=Y2T[:, ts], data=T2.rearrange("p a b -> p (a b)"),
                        idxs=IDX[:, ts], i_know_ap_gather_is_preferred=True)

                # per-row stat chains: one bf16 pair-halving + packed reduce
                def chain(dst, src, op):
                    H = hp.tile([128, BM, C // 2], bf16, tag="H")
                    nc.vector.tensor_tensor(
                        out=H, in0=src[:, :, 0:64], in1=src[:, :, 64:128], op=op)
                    nc.vector.tensor_reduce(out=dst, in_=H, axis=X, op=op)

                chain(S1[:, ts], E1, Alu.add)
                chain(S2[:, ts], E2, Alu.add)

                # A1 = sum (T1-T2)*E1, A2 = sum (T1-T2)*E2
                nc.vector.tensor_tensor(out=PD1, in0=D, in1=E1, op=Alu.mult)
                if m % 8 in (1, 2, 4, 5, 7):       # 10 of 16 macros on gpsimd
                    nc.gpsimd.tensor_tensor(out=PD2, in0=D, in1=E2, op=Alu.mult)
                else:
                    nc.vector.tensor_tensor(out=PD2, in0=D, in1=E2, op=Alu.mult)
                chain(A1[:, ts], PD1, Alu.add)
                chain(A2[:, ts], PD2, Alu.add)

            # ---------------- epilogue ----------------
            # Order matters per-engine: the selection counts go first so the
            # AllReduce launches ASAP; KL math and dumps fill its latency.
            MISC = ep.tile([128, 8], f32, tag="MISC")
            nc.vector.memset(MISC, 0.0)

            LZ1 = ep.tile([128, NT], f32, tag="LZ1")
            LZ2 = ep.tile([128, NT], f32, tag="LZ2")
            nc.scalar.activation(out=LZ1, in_=S1, func=Act.Ln)
            nc.scalar.activation(out=LZ2, in_=S2, func=Act.Ln)
            LZ12 = ep.tile([128, NT], f32, tag="LZ12")
            nc.vector.tensor_tensor(out=LZ12, in0=LZ1, in1=LZ2, op=Alu.add)
            Y12 = ep.tile([128, NT], f32, tag="Y12")
            nc.vector.tensor_tensor(out=Y12, in0=Y1T, in1=Y2T, op=Alu.add)
            TL = ep.tile([128, NT], f32, tag="TL")
            nc.vector.tensor_tensor(out=TL, in0=LZ12, in1=Y12, op=Alu.subtract)

            # --- distributed selection: counts vs fixed grid ---
            CNT = ep.tile([128, SEL_NTH], f32, tag="CNT")
            for j in range(SEL_NTH):
                cs = escr.tile([128, NT], f32, tag="cs")
                nc.vector.tensor_scalar(
                    out=cs, in0=TL, scalar1=thr[:, j:j + 1], scalar2=None,
                    op0=Alu.is_lt, op1=Alu.add, accum_out=CNT[:, j:j + 1])

            CNTP = ep.tile([128, SEL_NTH], f32, tag="CNTP")
            nc.gpsimd.partition_all_reduce(
                out_ap=CNTP, in_ap=CNT, channels=128,
                reduce_op=bass_isa.ReduceOp.add)

            cc_in = dp.tile([1, SEL_NTH], f32, tag="cc_in")
            cc_out = dp.tile([1, SEL_NTH], f32, tag="cc_out")
            nc.sync.dma_start(out=cc_in, in_=CNTP[0:1, :])
            nc.gpsimd.collective_compute(
                "AllReduce", Alu.add,
                replica_groups=[list(range(NCORES))],
                ins=[cc_in[:].opt()], outs=[cc_out[:].opt()])

            # CC-independent work fills the collective latency
            nc.sync.dma_start(out=o_tl, in_=TL)
            nc.vector.tensor_reduce(out=MISC[:, 3:4], in_=TL, axis=X, op=Alu.add)
            R1 = ep.tile([128, NT], f32, tag="R1")
            R2 = ep.tile([128, NT], f32, tag="R2")
            nc.vector.reciprocal(out=R1, in_=S1)
            nc.vector.reciprocal(out=R2, in_=S2)
            KA = ep.tile([128, NT], f32, tag="KA")
            KB = ep.tile([128, NT], f32, tag="KB")
            nc.vector.tensor_tensor(out=KA, in0=A1, in1=R1, op=Alu.mult)
            nc.vector.tensor_tensor(out=KB, in0=A2, in1=R2, op=Alu.mult)
            KL = ep.tile([128, NT], f32, tag="KL")
            nc.vector.tensor_tensor(out=KL, in0=KA, in1=KB, op=Alu.subtract)
            nc.vector.tensor_reduce(out=MISC[:, 2:3], in_=KL, axis=X, op=Alu.add)

            CNTG0 = ep.tile([1, SEL_NTH], f32, tag="CNTG0")
            nc.sync.dma_start(out=CNTG0, in_=cc_out)
            nc.sync.dma_start(out=o_cnt, in_=CNTG0)
            CNTG = ep.tile([128, SEL_NTH], f32, tag="CNTG")
            nc.gpsimd.partition_broadcast(out_ap=CNTG, in_ap=CNTG0, channels=128)

            # edge a = SEL_LO + s*W with s = #{j: cnt_j < k}
            EM = ep.tile([128, SEL_NTH], f32, tag="EM")
            nc.vector.tensor_scalar(
                out=EM, in0=CNTG, scalar1=kval[:, 0:1], scalar2=None,
                op0=Alu.is_lt)
            SIDX = ep.tile([128, 1], f32, tag="SIDX")
            nc.vector.tensor_reduce(out=SIDX, in_=EM, axis=X, op=Alu.add)
            AED = ep.tile([128, 1], f32, tag="AED")
            nc.vector.tensor_scalar(
                out=AED, in0=SIDX, scalar1=SEL_W, scalar2=SEL_LO,
                op0=Alu.mult, op1=Alu.add)
            AEDW = ep.tile([128, 1], f32, tag="AEDW")
            nc.vector.tensor_scalar(
                out=AEDW, in0=AED, scalar1=SEL_W, scalar2=None, op0=Alu.add)
            nc.vector.tensor_copy(out=MISC[:, 4:5], in_=AED)

            # exact n_below / S_below at edge a
            e1s = escr.tile([128, NT], f32, tag="cs")
            nc.vector.tensor_scalar(
                out=e1s, in0=TL, scalar1=AED[:, 0:1], scalar2=None,
                op0=Alu.is_lt, op1=Alu.add, accum_out=MISC[:, 0:1])
            e2s = escr.tile([128, NT], f32, tag="cs")
            nc.vector.scalar_tensor_tensor(
                out=e2s, in0=TL, scalar=AED[:, 0:1], in1=TL,
                op0=Alu.is_lt, op1=Alu.mult, accum_out=MISC[:, 1:2])

            nc.sync.dma_start(out=o_misc, in_=MISC)

    nc.compile()
    return nc


def _get_compiled():
    if "nc" not in _CACHE:
        _CACHE["nc"] = _build()
    return _CACHE["nc"]


def _host_inputs(y1, y2, targets):
    idx = _row_index_map()                      # [128, NT] local rows
    b_of_t = (np.arange(NT) % BM)[None, :]      # group within macro
    thr_row = (np.arange(1, SEL_NTH + 1, dtype=np.float32)
               * np.float32(SEL_W) + np.float32(SEL_LO))
    thr = np.broadcast_to(thr_row[None, :], (128, SEL_NTH)).copy()

    in_maps = []
    for cid in range(NCORES):
        lo = cid * SHARD
        tshard = np.asarray(targets[lo:lo + SHARD]).astype(np.int64)
        tgt = tshard[idx]                       # [128, NT]
        idx16 = (b_of_t * C + tgt).astype(np.uint16)
        in_maps.append({
            "y1s": np.ascontiguousarray(y1[lo:lo + SHARD]),
            "y2s": np.ascontiguousarray(y2[lo:lo + SHARD]),
            "idx16": idx16,
            "thr": thr,
            "kval": np.zeros((128, 1), np.float32),
        })
    return in_maps


def _host_finish(results, y1, y2, targets, epoch, k):
    n = N
    idx = _row_index_map()

    kl_sum = np.float64(0.0)
    s_total = np.float64(0.0)
    n_below = np.float64(0.0)
    s_below = np.float64(0.0)
    tl_full = np.empty(n, np.float32)
    fallback = False
    edge_a = None

    for cid, r in enumerate(results):
        misc = r["o_misc"].astype(np.float64)        # [128, 8]
        kl_sum += misc[:, 2].sum()
        s_total += misc[:, 3].sum()
        n_below += misc[:, 0].sum()
        s_below += misc[:, 1].sum()
        ea = r["o_misc"][0, 4]
        if edge_a is None:
            edge_a = ea
        elif ea != edge_a:
            fallback = True
        tl_core = r["o_tl"]                          # [128, NT]
        gl = cid * SHARD + idx
        tl_full[gl.ravel()] = tl_core.ravel()

    boundary = (np.sort(tl_full[(tl_full >= edge_a)
                                & (tl_full < edge_a + np.float32(SEL_W))])
                if edge_a is not None else np.empty(0, np.float32))

    if epoch == 0:
        return np.float32(s_total / n)

    need = k - int(round(n_below))
    if fallback or need < 0 or need > boundary.size:
        # safety net: exact selection on the dumped per-sample losses
        part = np.partition(tl_full, k - 1)
        tau = part[k - 1]
        below = tl_full < tau
        nb = int(below.sum())
        clean_sum = np.float64(tl_full[below].sum()) + (k - nb) * np.float64(tau)
    else:
        sel = boundary[:need]
        tau = sel[-1] if need > 0 else np.float32(edge_a)
        clean_sum = s_below + np.float64(sel.sum())

    clean_mean = clean_sum / k

    # corr term over the noisy set. Noisy rows all satisfy tl >= tau, a
    # tiny fraction of N; evaluate their agree/conf masks vectorized.
    corr_mean = np.float64(0.0)
    cand = np.nonzero(tl_full >= tau)[0]
    if cand.size:
        # resolve which candidates are actually noisy (stable-sort ties)
        vc = tl_full[cand]
        noisy_mask = vc > tau
        ties = np.nonzero(vc == tau)[0]
        if ties.size:
            nb_strict = int((tl_full < tau).sum())
            n_clean_ties = k - nb_strict
            tie_rows_all = np.nonzero(tl_full == tau)[0]
            pos = np.searchsorted(tie_rows_all, cand[ties])
            noisy_mask[ties] = pos >= n_clean_ties
        rows = cand[noisy_mask]
        if rows.size:
            a1 = y1[rows].astype(np.float64)
            a2 = y2[rows].astype(np.float64)
            m1 = a1.max(axis=1, keepdims=True)
            m2 = a2.max(axis=1, keepdims=True)
            e1 = np.exp(a1 - m1)
            e2 = np.exp(a2 - m2)
            s1 = e1.sum(axis=1, keepdims=True)
            s2 = e2.sum(axis=1, keepdims=True)
            p1 = e1 / s1
            p2 = e2 / s2
            pr1 = np.argmax(a1, axis=1)
            pr2 = np.argmax(a2, axis=1)
            conf = p1.max(axis=1) * p2.max(axis=1)
            mask = (pr1 == pr2) & (conf > 0.5)
            if mask.any():
                w = np.sqrt(conf[mask])
                sel1 = p1[mask, pr1[mask]]
                sel2 = p2[mask, pr1[mask]]
                corr = w * (-np.log(sel1) - np.log(sel2))
                corr_mean = np.float64(corr.sum()) / int(mask.sum())

    kl_loss = kl_sum / n
    return np.float32(clean_mean + corr_mean + CO_LAMBDA * kl_loss)


def kernel(**inputs):
    from concourse import bass_utils

    y1 = np.asarray(inputs["y1"], dtype=np.float32)
    y2 = np.asarray(inputs["y2"], dtype=np.float32)
    targets = np.asarray(inputs["targets"])
    epoch = int(np.asarray(inputs["epoch"]))

    forget_rate = min(0.5, INCREMENT * epoch)
    remember_rate = max(0.5, 1.0 - forget_rate)
    k = int(remember_rate * N)

    nc = _get_compiled()
    in_maps = _host_inputs(y1, y2, targets)
    for m in in_maps:
        m["kval"][:] = np.float32(k)

    res = bass_utils.run_bass_kernel_spmd(
        nc, in_maps, core_ids=list(range(NCORES)))
    results = res.results

    return np.array(_host_finish(results, y1, y2, targets, epoch, k),
                    dtype=np.float32)


# revision 18
# speedup vs baseline: 1.5381x; 1.1501x over previous
"""AdaptiveLoss (co-teaching style loss) Trainium2 kernel, 8 NeuronCores.

Matches the jax reference:
  per-sample CE of y1,y2 at targets -> total_loss; symmetric batchmean KL
  between softmax(y1) and softmax(y2); clean mean over the num_remember
  globally-smallest total_loss; correction term over the noisy set
  (empty for prod_conf<=0.5, which the device flags with a sound filter).

Per core (data-parallel over N, 32768 rows = 16 macro-tiles [128,16,128]):
  ACT    : E = exp(T) f32->bf16, one op per macro-tensor
  DVE    : row maxes (packed reduce), bf16 products (T1-T2)*E with
           pair-halving adds, packed segmented reduces for s1,s2,A1,A2
  GPSIMD : D = T1-T2 (most macros), per-row target gathers (indirect_copy)
  kl_i = A1/s1 - A2/s2 ; total_loss_i = ln(s1)+ln(s2) - y1[t] - y2[t]

Global k-th smallest: 32-edge fixed grid counts (tensor_scalar+accum),
one AllReduce, exact below-edge count/sum at the picked edge, boundary
values extracted with sparse_gather; host sorts the tiny boundary set and
finishes the scalar (plus exact corr fix-up for flagged rows, and an
exact fallback from the dumped per-sample losses if the grid was missed).
"""

import numpy as np

N, C = 262144, 128
NCORES = 8
SHARD = N // NCORES            # 32768 rows per core
NT = SHARD // 128              # 256 row-tiles per core
BM = 16                        # tiles per macro-tile
NMACRO = NT // BM              # 16
EPOCHS = 100
CO_LAMBDA = 0.1
INCREMENT = 0.5 / EPOCHS

# selection grid: 32 dyadic edges over (SEL_LO, SEL_LO + 32*SEL_W]
SEL_LO = 12.9
SEL_W = 0.125                  # 2^-3, exact in f32; span (12.9, 14.9]
SEL_NTH = 16
BV_CAP = 512                   # sparse_gather out free size (16*512 values)
GPS_D_MACROS = 0              # macros whose D runs on gpsimd (rest on DVE)

_CACHE = {}


def _row_index_map():
    """(p, t) -> local row index. Macro m covers rows [2048m, 2048(m+1));
    partition p holds rows 2048m + 16p + b; stats column t = m*BM + b."""
    p = np.arange(128)[:, None]
    t = np.arange(NT)[None, :]
    m = t // BM
    b = t % BM
    return (2048 * m + 16 * p + b).astype(np.int64)  # [128, NT]


import os
DISABLE = set(os.environ.get('KDISABLE', '').split(','))


def _build():
    import concourse.bass as bass
    import concourse.bacc as bacc
    import concourse.tile as tile
    import concourse.bass_isa as bass_isa
    from concourse import mybir

    f32 = mybir.dt.float32
    bf16 = mybir.dt.bfloat16
    u32 = mybir.dt.uint32
    u16 = mybir.dt.uint16
    Alu = mybir.AluOpType
    Act = mybir.ActivationFunctionType
    X = mybir.AxisListType.X

    nc = bacc.Bacc("TRN2", target_bir_lowering=False, debug=False,
                   num_devices=NCORES)

    y1 = nc.dram_tensor("y1s", [SHARD, C], f32, kind="ExternalInput").ap()
    y2 = nc.dram_tensor("y2s", [SHARD, C], f32, kind="ExternalInput").ap()
    idx_d = nc.dram_tensor("idx16", [128, NT], u16, kind="ExternalInput").ap()
    thr_d = nc.dram_tensor("thr", [128, SEL_NTH], f32, kind="ExternalInput").ap()
    kval_d = nc.dram_tensor("kval", [128, 1], f32, kind="ExternalInput").ap()

    o_tl = nc.dram_tensor("o_tl", [128, NT], f32, kind="ExternalOutput").ap()
    o_misc = nc.dram_tensor("o_misc", [128, 8], f32, kind="ExternalOutput").ap()
    o_cnt = nc.dram_tensor("o_cnt", [1, SEL_NTH], f32, kind="ExternalOutput").ap()

    y1v = y1.rearrange("(m p b) c -> m p b c", m=NMACRO, p=128)
    y2v = y2.rearrange("(m p b) c -> m p b c", m=NMACRO, p=128)

    with tile.TileContext(nc) as tc:
        with (
            tc.tile_pool(name="io", bufs=3) as iop,
            tc.tile_pool(name="work", bufs=3) as wp,
            tc.tile_pool(name="half", bufs=4) as hp,
            tc.tile_pool(name="stats", bufs=1) as sp,
            tc.tile_pool(name="epi", bufs=1) as ep,
            tc.tile_pool(name="escr", bufs=2) as escr,
            tc.tile_pool(name="dram", bufs=1, space="DRAM") as dp,
        ):
            S1 = sp.tile([128, NT], f32, tag="S1")
            S2 = sp.tile([128, NT], f32, tag="S2")
            A1 = sp.tile([128, NT], f32, tag="A1")
            A2 = sp.tile([128, NT], f32, tag="A2")
            Y1T = sp.tile([128, NT], f32, tag="Y1T")
            Y2T = sp.tile([128, NT], f32, tag="Y2T")
            IDX = sp.tile([128, NT], u16, tag="IDX")
            thr = sp.tile([128, SEL_NTH], f32, tag="thr")
            kval = sp.tile([128, 1], f32, tag="kval")
            nc.sync.dma_start(out=IDX, in_=idx_d)
            nc.sync.dma_start(out=thr, in_=thr_d)
            nc.sync.dma_start(out=kval, in_=kval_d)

            # ---------------- streaming phase ----------------
            for m in range(NMACRO):
                ts = slice(m * BM, (m + 1) * BM)
                T1 = iop.tile([128, BM, C], f32, tag="T1")
                T2 = iop.tile([128, BM, C], f32, tag="T2")
                nc.sync.dma_start(out=T1, in_=y1v[m])
                nc.sync.dma_start(out=T2, in_=y2v[m])

                E1 = wp.tile([128, BM, C], bf16, tag="E1")
                E2 = wp.tile([128, BM, C], bf16, tag="E2")
                D = wp.tile([128, BM, C], bf16, tag="D")
                PD1 = wp.tile([128, BM, C], bf16, tag="PD1")
                PD2 = wp.tile([128, BM, C], bf16, tag="PD2")

                # exps (one big ACT op each)
                nc.scalar.activation(out=E1, in_=T1, func=Act.Exp)
                nc.scalar.activation(out=E2, in_=T2, func=Act.Exp)

                # D = T1 - T2 (bf16 out)
                if m < GPS_D_MACROS:
                    nc.gpsimd.tensor_tensor(out=D, in0=T1, in1=T2, op=Alu.subtract)
                else:
                    nc.vector.tensor_tensor(out=D, in0=T1, in1=T2, op=Alu.subtract)

                # target gathers: Y[:, t] = T[p, idx[p, t]] (gpsimd software)
                if "gather" in DISABLE:
                    nc.vector.memset(Y1T[:, ts], 5.0)
                    nc.vector.memset(Y2T[:, ts], 5.0)
                else:
                    nc.gpsimd.indirect_copy(
                        out=Y1T[:, ts], data=T1.rearrange("p a b -> p (a b)"),
                        idxs=IDX[:, ts], i_know_ap_gather_is_preferred=True)
                    nc.gpsimd.indirect_copy(
                        out=Y2T[:, ts], data=T2.rearrange("p a b -> p (a b)"),
                        idxs=IDX[:, ts], i_know_ap_gather_is_preferred=True)

                # per-row stat chains: one bf16 pair-halving + packed reduce
                def chain(dst, src, op):
                    H = hp.tile([128, BM, C // 2], bf16, tag="H")
                    nc.vector.tensor_tensor(
                        out=H, in0=src[:, :, 0:64], in1=src[:, :, 64:128], op=op)
                    nc.vector.tensor_reduce(out=dst, in_=H, axis=X, op=op)

                chain(S1[:, ts], E1, Alu.add)
                chain(S2[:, ts], E2, Alu.add)

                # A1 = sum (T1-T2)*E1, A2 = sum (T1-T2)*E2
                nc.vector.tensor_tensor(out=PD1, in0=D, in1=E1, op=Alu.mult)
                nc.vector.tensor_tensor(out=PD2, in0=D, in1=E2, op=Alu.mult)
                chain(A1[:, ts], PD1, Alu.add)
                chain(A2[:, ts], PD2, Alu.add)

            # ---------------- epilogue ----------------
            # Order matters per-engine: the selection counts go first so the
            # AllReduce launches ASAP; KL math and dumps fill its latency.
            MISC = ep.tile([128, 8], f32, tag="MISC")
            nc.vector.memset(MISC, 0.0)

            LZ1 = ep.tile([128, NT], f32, tag="LZ1")
            LZ2 = ep.tile([128, NT], f32, tag="LZ2")
            nc.scalar.activation(out=LZ1, in_=S1, func=Act.Ln)
            nc.scalar.activation(out=LZ2, in_=S2, func=Act.Ln)
            LZ12 = ep.tile([128, NT], f32, tag="LZ12")
            nc.vector.tensor_tensor(out=LZ12, in0=LZ1, in1=LZ2, op=Alu.add)
            Y12 = ep.tile([128, NT], f32, tag="Y12")
            nc.vector.tensor_tensor(out=Y12, in0=Y1T, in1=Y2T, op=Alu.add)
            TL = ep.tile([128, NT], f32, tag="TL")
            nc.vector.tensor_tensor(out=TL, in0=LZ12, in1=Y12, op=Alu.subtract)

            # --- distributed selection: counts vs fixed grid ---
            CNT = ep.tile([128, SEL_NTH], f32, tag="CNT")
            for j in range(SEL_NTH):
                cs = escr.tile([128, NT], f32, tag="cs")
                nc.vector.tensor_scalar(
                    out=cs, in0=TL, scalar1=thr[:, j:j + 1], scalar2=None,
                    op0=Alu.is_lt, op1=Alu.add, accum_out=CNT[:, j:j + 1])

            CNTP = ep.tile([128, SEL_NTH], f32, tag="CNTP")
            nc.gpsimd.partition_all_reduce(
                out_ap=CNTP, in_ap=CNT, channels=128,
                reduce_op=bass_isa.ReduceOp.add)

            cc_in = dp.tile([1, SEL_NTH], f32, tag="cc_in")
            cc_out = dp.tile([1, SEL_NTH], f32, tag="cc_out")
            nc.sync.dma_start(out=cc_in, in_=CNTP[0:1, :])
            nc.gpsimd.collective_compute(
                "AllReduce", Alu.add,
                replica_groups=[list(range(NCORES))],
                ins=[cc_in[:].opt()], outs=[cc_out[:].opt()])

            # CC-independent work fills the collective latency
            nc.sync.dma_start(out=o_tl, in_=TL)
            nc.vector.tensor_reduce(out=MISC[:, 3:4], in_=TL, axis=X, op=Alu.add)
            R1 = ep.tile([128, NT], f32, tag="R1")
            R2 = ep.tile([128, NT], f32, tag="R2")
            nc.vector.reciprocal(out=R1, in_=S1)
            nc.vector.reciprocal(out=R2, in_=S2)
            KA = ep.tile([128, NT], f32, tag="KA")
            KB = ep.tile([128, NT], f32, tag="KB")
            nc.vector.tensor_tensor(out=KA, in0=A1, in1=R1, op=Alu.mult)
            nc.vector.tensor_tensor(out=KB, in0=A2, in1=R2, op=Alu.mult)
            KL = ep.tile([128, NT], f32, tag="KL")
            nc.vector.tensor_tensor(out=KL, in0=KA, in1=KB, op=Alu.subtract)
            nc.vector.tensor_reduce(out=MISC[:, 2:3], in_=KL, axis=X, op=Alu.add)

            CNTG0 = ep.tile([1, SEL_NTH], f32, tag="CNTG0")
            nc.sync.dma_start(out=CNTG0, in_=cc_out)
            nc.sync.dma_start(out=o_cnt, in_=CNTG0)
            CNTG = ep.tile([128, SEL_NTH], f32, tag="CNTG")
            nc.gpsimd.partition_broadcast(out_ap=CNTG, in_ap=CNTG0, channels=128)

            # edge a = SEL_LO + s*W with s = #{j: cnt_j < k}
            EM = ep.tile([128, SEL_NTH], f32, tag="EM")
            nc.vector.tensor_scalar(
                out=EM, in0=CNTG, scalar1=kval[:, 0:1], scalar2=None,
                op0=Alu.is_lt)
            SIDX = ep.tile([128, 1], f32, tag="SIDX")
            nc.vector.tensor_reduce(out=SIDX, in_=EM, axis=X, op=Alu.add)
            AED = ep.tile([128, 1], f32, tag="AED")
            nc.vector.tensor_scalar(
                out=AED, in0=SIDX, scalar1=SEL_W, scalar2=SEL_LO,
                op0=Alu.mult, op1=Alu.add)
            AEDW = ep.tile([128, 1], f32, tag="AEDW")
            nc.vector.tensor_scalar(
                out=AEDW, in0=AED, scalar1=SEL_W, scalar2=None, op0=Alu.add)
            nc.vector.tensor_copy(out=MISC[:, 4:5], in_=AED)

            # exact n_below / S_below at edge a
            e1s = escr.tile([128, NT], f32, tag="cs")
            nc.vector.tensor_scalar(
                out=e1s, in0=TL, scalar1=AED[:, 0:1], scalar2=None,
                op0=Alu.is_lt, op1=Alu.add, accum_out=MISC[:, 0:1])
            e2s = escr.tile([128, NT], f32, tag="cs")
            nc.vector.scalar_tensor_tensor(
                out=e2s, in0=TL, scalar=AED[:, 0:1], in1=TL,
                op0=Alu.is_lt, op1=Alu.mult, accum_out=MISC[:, 1:2])

            nc.sync.dma_start(out=o_misc, in_=MISC)

    nc.compile()
    return nc


def _get_compiled():
    if "nc" not in _CACHE:
        _CACHE["nc"] = _build()
    return _CACHE["nc"]


def _host_inputs(y1, y2, targets):
    idx = _row_index_map()                      # [128, NT] local rows
    b_of_t = (np.arange(NT) % BM)[None, :]      # group within macro
    thr_row = (np.arange(1, SEL_NTH + 1, dtype=np.float32)
               * np.float32(SEL_W) + np.float32(SEL_LO))
    thr = np.broadcast_to(thr_row[None, :], (128, SEL_NTH)).copy()

    in_maps = []
    for cid in range(NCORES):
        lo = cid * SHARD
        tshard = np.asarray(targets[lo:lo + SHARD]).astype(np.int64)
        tgt = tshard[idx]                       # [128, NT]
        idx16 = (b_of_t * C + tgt).astype(np.uint16)
        in_maps.append({
            "y1s": np.ascontiguousarray(y1[lo:lo + SHARD]),
            "y2s": np.ascontiguousarray(y2[lo:lo + SHARD]),
            "idx16": idx16,
            "thr": thr,
            "kval": np.zeros((128, 1), np.float32),
        })
    return in_maps


def _host_finish(results, y1, y2, targets, epoch, k):
    n = N
    idx = _row_index_map()

    kl_sum = np.float64(0.0)
    s_total = np.float64(0.0)
    n_below = np.float64(0.0)
    s_below = np.float64(0.0)
    tl_full = np.empty(n, np.float32)
    fallback = False
    edge_a = None

    for cid, r in enumerate(results):
        misc = r["o_misc"].astype(np.float64)        # [128, 8]
        kl_sum += misc[:, 2].sum()
        s_total += misc[:, 3].sum()
        n_below += misc[:, 0].sum()
        s_below += misc[:, 1].sum()
        ea = r["o_misc"][0, 4]
        if edge_a is None:
            edge_a = ea
        elif ea != edge_a:
            fallback = True
        tl_core = r["o_tl"]                          # [128, NT]
        gl = cid * SHARD + idx
        tl_full[gl.ravel()] = tl_core.ravel()

    boundary = (np.sort(tl_full[(tl_full >= edge_a)
                                & (tl_full < edge_a + np.float32(SEL_W))])
                if edge_a is not None else np.empty(0, np.float32))

    if epoch == 0:
        return np.float32(s_total / n)

    need = k - int(round(n_below))
    if fallback or need < 0 or need > boundary.size:
        # safety net: exact selection on the dumped per-sample losses
        part = np.partition(tl_full, k - 1)
        tau = part[k - 1]
        below = tl_full < tau
        nb = int(below.sum())
        clean_sum = np.float64(tl_full[below].sum()) + (k - nb) * np.float64(tau)
    else:
        sel = boundary[:need]
        tau = sel[-1] if need > 0 else np.float32(edge_a)
        clean_sum = s_below + np.float64(sel.sum())

    clean_mean = clean_sum / k

    # corr term over the noisy set. Noisy rows all satisfy tl >= tau, a
    # tiny fraction of N; evaluate their agree/conf masks vectorized.
    corr_mean = np.float64(0.0)
    cand = np.nonzero(tl_full >= tau)[0]
    if cand.size:
        # resolve which candidates are actually noisy (stable-sort ties)
        vc = tl_full[cand]
        noisy_mask = vc > tau
        ties = np.nonzero(vc == tau)[0]
        if ties.size:
            nb_strict = int((tl_full < tau).sum())
            n_clean_ties = k - nb_strict
            tie_rows_all = np.nonzero(tl_full == tau)[0]
            pos = np.searchsorted(tie_rows_all, cand[ties])
            noisy_mask[ties] = pos >= n_clean_ties
        rows = cand[noisy_mask]
        if rows.size:
            a1 = y1[rows].astype(np.float64)
            a2 = y2[rows].astype(np.float64)
            m1 = a1.max(axis=1, keepdims=True)
            m2 = a2.max(axis=1, keepdims=True)
            e1 = np.exp(a1 - m1)
            e2 = np.exp(a2 - m2)
            s1 = e1.sum(axis=1, keepdims=True)
            s2 = e2.sum(axis=1, keepdims=True)
            p1 = e1 / s1
            p2 = e2 / s2
            pr1 = np.argmax(a1, axis=1)
            pr2 = np.argmax(a2, axis=1)
            conf = p1.max(axis=1) * p2.max(axis=1)
            mask = (pr1 == pr2) & (conf > 0.5)
            if mask.any():
                w = np.sqrt(conf[mask])
                sel1 = p1[mask, pr1[mask]]
                sel2 = p2[mask, pr1[mask]]
                corr = w * (-np.log(sel1) - np.log(sel2))
                corr_mean = np.float64(corr.sum()) / int(mask.sum())

    kl_loss = kl_sum / n
    return np.float32(clean_mean + corr_mean + CO_LAMBDA * kl_loss)


def kernel(**inputs):
    from concourse import bass_utils

    y1 = np.asarray(inputs["y1"], dtype=np.float32)
    y2 = np.asarray(inputs["y2"], dtype=np.float32)
    targets = np.asarray(inputs["targets"])
    epoch = int(np.asarray(inputs["epoch"]))

    forget_rate = min(0.5, INCREMENT * epoch)
    remember_rate = max(0.5, 1.0 - forget_rate)
    k = int(remember_rate * N)

    nc = _get_compiled()
    in_maps = _host_inputs(y1, y2, targets)
    for m in in_maps:
        m["kval"][:] = np.float32(k)

    res = bass_utils.run_bass_kernel_spmd(
        nc, in_maps, core_ids=list(range(NCORES)))
    results = res.results

    return np.array(_host_finish(results, y1, y2, targets, epoch, k),
                    dtype=np.float32)
